# revision 24
# baseline (speedup 1.0000x reference)
"""Multi-head causal attention (B=4,T=2048,C=1024,H=16,HS=64) on 8 TRN2 cores.

Sharding: core c -> batch b=c//2, head-group hg=c%2 (8 heads each).
Each core computes QKV projections for its heads, causal flash-attention,
and a partial output projection over its 512 combo channels, emitting
out^T partial [1024, 2048].  Host sums the two partials per batch (the
tensor-parallel all-reduce) and transposes.

Matmuls run in bf16 (fp32 PSUM accumulation); softmax skips
max-subtraction (scores ~ N(0,1), exp never overflows); the softmax
denominator comes free as a 65th row of the PV matmul via a ones-column
appended to V.  The kernel is a single software pipeline over the four
512-query blocks: QKV(tb) -> attention(qb=tb, all heads) -> proj(tb).
"""

import os
import sys

if "/opt/trn_rl_repo" not in sys.path:
    sys.path.insert(0, "/opt/trn_rl_repo")

import ml_dtypes
import numpy as np

import concourse.bass as bass
import concourse.mybir as mybir
import concourse.tile as tile
from concourse import bacc
from concourse.bass_utils import run_bass_kernel_spmd

P = 128
B, T, C, H = 4, 2048, 1024, 16
HS = C // H              # 64
HL = H // 2              # 8 local heads per core
HD = HL * HS             # 512 local combo channels
NT = T // 512            # 4 query blocks of 512
NCC = C // P             # 8 contraction chunks over C
NKC = T // P             # 16 key chunks of 128
F32 = mybir.dt.float32
F32R = mybir.dt.float32r
BF16 = mybir.dt.bfloat16
EXP_SCALE = float(HS) ** -0.5  # 1/8, folded into the exp activation

# matmul input dtype: bf16 (full PE rate @2.4GHz) or f32r (~1.2GHz, 10x tighter)
MM_DT = {"bf16": BF16, "f32r": F32R}[os.environ.get("MM_DT", "bf16")]
MM_NP = {BF16: ml_dtypes.bfloat16, F32R: np.float32}[MM_DT]

_PROGRAM = None


def _build_program():
    nc = bacc.Bacc("TRN2", target_bir_lowering=False, debug=False, num_devices=8)

    xT = nc.dram_tensor("xT", [C, T], MM_DT, kind="ExternalInput")
    wq = nc.dram_tensor("wq", [C, HD], MM_DT, kind="ExternalInput")
    wk = nc.dram_tensor("wk", [C, HD], MM_DT, kind="ExternalInput")
    wv = nc.dram_tensor("wv", [C, HD], MM_DT, kind="ExternalInput")
    wpT = nc.dram_tensor("wpT", [HD, C], MM_DT, kind="ExternalInput")
    bias = nc.dram_tensor("bias", [C], F32, kind="ExternalInput")
    masks = nc.dram_tensor("masks", [4, P, 512], MM_DT, kind="ExternalInput")
    onescol = nc.dram_tensor("onescol", [P, NKC, HL], MM_DT, kind="ExternalInput")
    outT = nc.dram_tensor("outT", [C, T], F32, kind="ExternalOutput")

    Exp = mybir.ActivationFunctionType.Exp

    with tile.TileContext(nc) as tc:
        with (
            tc.tile_pool(name="persist", bufs=1) as persist,
            tc.tile_pool(name="xtp", bufs=2) as xtp,
            tc.tile_pool(name="ctp", bufs=2) as ctp,
            tc.tile_pool(name="ptp", bufs=2) as ptp,
            tc.tile_pool(name="misc", bufs=2) as misc,
            tc.tile_pool(name="outp", bufs=3) as outp,
            tc.tile_pool(name="ps_gen", bufs=2, space="PSUM") as ps_gen,
            tc.tile_pool(name="ps_s", bufs=1, space="PSUM") as ps_s,
            tc.tile_pool(name="ps_o", bufs=2, space="PSUM") as ps_o,
        ):
            # Q^T / K^T with head pairs stacked on partitions: chunk j holds
            # head 2j in rows 0-63 and head 2j+1 in rows 64-127
            qt = persist.tile([P, HL // 2, T], MM_DT)
            kt = persist.tile([P, HL // 2, T], MM_DT)
            vaug = persist.tile([P, NKC, HL, HS + 1], MM_DT)
            bias_sb = persist.tile([P, C // P], F32)
            wq_sb = persist.tile([P, NCC, HD], MM_DT, tag="wq")
            wk_sb = persist.tile([P, NCC, HD], MM_DT, tag="wk")
            wv_sb = persist.tile([P, NCC, HD], MM_DT, tag="wv")
            wpT_sb = persist.tile([P, HD // P, C], MM_DT, tag="wpT")
            masks_sb = persist.tile([P, 4, 512], MM_DT, tag="masks")

            nc.sync.dma_start(bias_sb[:], bias[:].rearrange("(db p) -> p db", p=P))
            # ones column for the softmax-denominator row of the PV matmul
            nc.sync.dma_start(vaug[:, :, :, HS : HS + 1], onescol[:, :, :, None])
            nc.sync.dma_start(wq_sb[:], wq[:].rearrange("(co p) n -> p co n", p=P))
            nc.sync.dma_start(wk_sb[:], wk[:].rearrange("(co p) n -> p co n", p=P))
            nc.sync.dma_start(wv_sb[:], wv[:].rearrange("(co p) n -> p co n", p=P))
            nc.sync.dma_start(masks_sb[:], masks[:].rearrange("i k q -> k i q"))
            nc.sync.dma_start(wpT_sb[:], wpT[:].rearrange("(co p) n -> p co n", p=P))

            # one persistent 4-bank scores psum: diagonal-suffix matmuls
            # intentionally leave stale (bounded) data in masked columns,
            # which same-tensor reuse keeps visible to the dep tracker
            pss = ps_s.tile([P, 4, 512], F32, tag="pss")

            for tb in range(NT):
                q0 = tb * 512
                tsl = slice(q0, q0 + 512)

                # ---- QKV projections for this t-block ----
                xt = xtp.tile([P, NCC, 512], MM_DT, tag="xt")
                nc.sync.dma_start(
                    xt[:], xT[:].rearrange("(co p) t -> p co t", p=P)[:, :, tsl]
                )
                for hb in range(HL // 2):
                    hsl = slice(hb * P, (hb + 1) * P)
                    for w_sb, dst in ((wq_sb, qt), (wk_sb, kt)):
                        pqk = ps_gen.tile([P, 512], F32, tag="pqk")
                        for co in range(NCC):
                            nc.tensor.matmul(
                                pqk[:],
                                w_sb[:, co, hsl],
                                xt[:, co, :],
                                start=(co == 0),
                                stop=(co == NCC - 1),
                            )
                        nc.vector.tensor_copy(out=dst[:, hb, tsl], in_=pqk[:])
                for ts2 in range(4):
                    pv = ps_gen.tile([P, 512], F32, tag="pqk")
                    for co in range(NCC):
                        nc.tensor.matmul(
                            pv[:],
                            xt[:, co, ts2 * P : (ts2 + 1) * P],
                            wv_sb[:, co, :],
                            start=(co == 0),
                            stop=(co == NCC - 1),
                        )
                    kc = tb * 4 + ts2
                    nc.vector.tensor_copy(
                        out=vaug[:, kc, :, 0:HS],
                        in_=pv[:].rearrange("p (h d) -> p h d", h=HL),
                    )

                # ---- causal attention for query block qb=tb, all heads ----
                comboT = ctp.tile([P, HD // P, 512], MM_DT, tag="comboT")
                last_kc = tb * 4 + 3
                for h in range(HL):
                    r0 = 64 * (h % 2)
                    hp = h // 2
                    po = ps_o.tile([P, 512], F32, tag="po")
                    for g in range(tb + 1):
                        for i in range(4):
                            kc = 4 * g + i
                            # diagonal tiles: columns q < kc*128-q0 are fully
                            # masked; skip them (psum there holds bounded
                            # stale scores — exp'd then zeroed by the mask).
                            # Only when tb>0 so the first-ever write of each
                            # bank is full-width.
                            c0 = kc * P - q0 if (g == tb and tb > 0) else 0
                            nc.tensor.matmul(
                                pss[:, i, c0:512],
                                kt[r0 : r0 + 64, hp, kc * P : (kc + 1) * P],
                                qt[r0 : r0 + 64, hp, q0 + c0 : q0 + 512],
                                start=True,
                                stop=True,
                                tile_position=(r0, 0),
                            )
                        pt = ptp.tile([P, 4, 512], MM_DT, tag="pt")
                        nc.scalar.activation(pt[:], pss[:], Exp, scale=EXP_SCALE)
                        if g == tb:
                            for i in range(4):
                                nc.vector.tensor_mul(
                                    out=pt[:, i, :],
                                    in0=pt[:, i, :],
                                    in1=masks_sb[:, i, :],
                                )
                        for i in range(4):
                            kc = 4 * g + i
                            # diagonal tiles: PT columns q < kc*128-q0 are
                            # zero (masked) — skip accumulating them
                            c0 = max(0, kc * P - q0) if g == tb else 0
                            nc.tensor.matmul(
                                po[0 : HS + 1, c0:512],
                                vaug[:, kc, h, :],
                                pt[:, i, c0:512],
                                start=(kc == 0),
                                stop=(kc == last_kc),
                            )
                    # normalize rows by the denominator row (65th) of po;
                    # custom-DVE reciprocal requires partition-0 input, so
                    # stage the row via an ACT copy first
                    den = misc.tile([1, 512], F32, tag="den")
                    nc.scalar.copy(out=den[:], in_=po[HS : HS + 1, :])
                    rc = misc.tile([1, 512], F32, tag="rc")
                    nc.vector.reciprocal_approx_fast(rc[:], den[:])
                    rb = misc.tile([HS, 512], F32, tag="rb")
                    nc.gpsimd.partition_broadcast(rb[:], rc[:])
                    nc.vector.tensor_mul(
                        out=comboT[(h % 2) * 64 : (h % 2) * 64 + 64, h // 2, :],
                        in0=po[0:HS, :],
                        in1=rb[:],
                    )

                # ---- partial output projection: out^T [d 128, t 512] ----
                for db in range(C // P):
                    pp = ps_gen.tile([P, 512], F32, tag="pqk")
                    for co in range(HD // P):
                        nc.tensor.matmul(
                            pp[:],
                            wpT_sb[:, co, db * P : (db + 1) * P],
                            comboT[:, co, :],
                            start=(co == 0),
                            stop=(co == HD // P - 1),
                        )
                    ot = outp.tile([P, 512], F32, tag="ot")
                    nc.vector.tensor_scalar_add(ot[:], pp[:], bias_sb[:, db : db + 1])
                    nc.sync.dma_start(outT[db * P : (db + 1) * P, tsl], ot[:])

    nc.finalize()
    return nc


def _causal_masks():
    # mask_i[kl, ql] = 1.0 iff (128*i + kl) <= ql, for the 4 diagonal key
    # chunks of a 512-wide query block (applied multiplicatively post-exp)
    kl = np.arange(P)[None, :, None]
    ql = np.arange(512)[None, None, :]
    i = np.arange(4)[:, None, None]
    return ((P * i + kl) <= ql).astype(np.float32)


def _in_maps(x, Wq, Wk, Wv, Wproj, bproj):
    masks = _causal_masks()
    zeros_bias = np.zeros_like(bproj)
    onescol = np.ones((P, NKC, HL), dtype=MM_NP)
    maps = []
    for core in range(8):
        b, hg = core // 2, core % 2
        hs = slice(hg * HL, (hg + 1) * HL)
        maps.append(
            {
                "xT": np.ascontiguousarray(x[b].T).astype(MM_NP),
                "wq": np.ascontiguousarray(
                    Wq[hs].transpose(1, 0, 2).reshape(C, HD).astype(MM_NP)
                ),
                "wk": np.ascontiguousarray(
                    Wk[hs].transpose(1, 0, 2).reshape(C, HD).astype(MM_NP)
                ),
                "wv": np.ascontiguousarray(
                    Wv[hs].transpose(1, 0, 2).reshape(C, HD).astype(MM_NP)
                ),
                "wpT": np.ascontiguousarray(Wproj[:, hg * HD : (hg + 1) * HD].T).astype(MM_NP),
                "bias": np.ascontiguousarray(bproj if hg == 0 else zeros_bias),
                "masks": masks.astype(MM_NP),
                "onescol": onescol,
            }
        )
    return maps


def get_program():
    global _PROGRAM
    if _PROGRAM is None:
        _PROGRAM = _build_program()
    return _PROGRAM


def kernel(x, Wq, Wk, Wv, Wproj, bproj, _run_kwargs=None):
    x = np.asarray(x, dtype=np.float32)
    Wq = np.asarray(Wq, dtype=np.float32)
    Wk = np.asarray(Wk, dtype=np.float32)
    Wv = np.asarray(Wv, dtype=np.float32)
    Wproj = np.asarray(Wproj, dtype=np.float32)
    bproj = np.asarray(bproj, dtype=np.float32)

    nc = get_program()
    res = run_bass_kernel_spmd(
        nc,
        _in_maps(x, Wq, Wk, Wv, Wproj, bproj),
        core_ids=list(range(8)),
        **(_run_kwargs or {}),
    )
    out = np.empty((B, T, C), dtype=np.float32)
    for b in range(B):
        out[b] = (res.results[2 * b]["outT"] + res.results[2 * b + 1]["outT"]).T
    if _run_kwargs:
        kernel.last_results = res
    return out


# revision 26
# speedup vs baseline: 1.3585x; 1.3585x over previous
"""Multi-head causal attention (B=4,T=2048,C=1024,H=16,HS=64) on 8 TRN2 cores.

Sharding: core c -> batch b=c//2, head-group hg=c%2 (8 heads each).
Each core computes QKV projections for its heads, causal flash-attention,
and a partial output projection over its 512 combo channels, emitting
out^T partial [1024, 2048].  Host sums the two partials per batch (the
tensor-parallel all-reduce) and transposes.

Matmuls run in bf16 (fp32 PSUM accumulation); softmax skips
max-subtraction (scores ~ N(0,1), exp never overflows); the softmax
denominator comes free as a 65th row of the PV matmul via a ones-column
appended to V.  The kernel is a single software pipeline over the four
512-query blocks: QKV(tb) -> attention(qb=tb, all heads) -> proj(tb).
"""

import os
import sys

if "/opt/trn_rl_repo" not in sys.path:
    sys.path.insert(0, "/opt/trn_rl_repo")

import ml_dtypes
import numpy as np

import concourse.bass as bass
import concourse.mybir as mybir
import concourse.tile as tile
from concourse import bacc
from concourse.bass_utils import run_bass_kernel_spmd

P = 128
B, T, C, H = 4, 2048, 1024, 16
HS = C // H              # 64
HL = H // 2              # 8 local heads per core
HD = HL * HS             # 512 local combo channels
NT = T // 512            # 4 query blocks of 512
NCC = C // P             # 8 contraction chunks over C
NKC = T // P             # 16 key chunks of 128
F32 = mybir.dt.float32
F32R = mybir.dt.float32r
BF16 = mybir.dt.bfloat16
EXP_SCALE = float(HS) ** -0.5  # 1/8, folded into the exp activation

# matmul input dtype: bf16 (full PE rate @2.4GHz) or f32r (~1.2GHz, 10x tighter)
MM_DT = {"bf16": BF16, "f32r": F32R}[os.environ.get("MM_DT", "bf16")]
MM_NP = {BF16: ml_dtypes.bfloat16, F32R: np.float32}[MM_DT]

_PROGRAM = None


def _build_program():
    nc = bacc.Bacc("TRN2", target_bir_lowering=False, debug=False, num_devices=8)

    xT = nc.dram_tensor("xT", [C, T], MM_DT, kind="ExternalInput")
    wq = nc.dram_tensor("wq", [C, HD], MM_DT, kind="ExternalInput")
    wk = nc.dram_tensor("wk", [C, HD], MM_DT, kind="ExternalInput")
    wv = nc.dram_tensor("wv", [C, HD], MM_DT, kind="ExternalInput")
    wpT = nc.dram_tensor("wpT", [HD, C], MM_DT, kind="ExternalInput")
    bias = nc.dram_tensor("bias", [C], F32, kind="ExternalInput")
    masks = nc.dram_tensor("masks", [4, P, 512], MM_DT, kind="ExternalInput")
    onescol = nc.dram_tensor("onescol", [P, NKC, HL], MM_DT, kind="ExternalInput")
    outT = nc.dram_tensor("outT", [C, T], F32, kind="ExternalOutput")

    Exp = mybir.ActivationFunctionType.Exp

    with tile.TileContext(nc) as tc:
        with (
            tc.tile_pool(name="persist", bufs=1) as persist,
            tc.tile_pool(name="xtp", bufs=2) as xtp,
                        tc.tile_pool(name="ptp", bufs=2) as ptp,
            tc.tile_pool(name="misc", bufs=2) as misc,
            tc.tile_pool(name="outp", bufs=3) as outp,
            tc.tile_pool(name="ps_gen", bufs=2, space="PSUM") as ps_gen,
            tc.tile_pool(name="ps_s", bufs=1, space="PSUM") as ps_s,
            tc.tile_pool(name="ps_o", bufs=2, space="PSUM") as ps_o,
        ):
            # Q^T / K^T with head pairs stacked on partitions: chunk j holds
            # head 2j in rows 0-63 and head 2j+1 in rows 64-127
            qt = persist.tile([P, HL // 2, T], MM_DT)
            kt = persist.tile([P, HL // 2, T], MM_DT)
            vaug = persist.tile([P, NKC, HL, HS + 1], MM_DT)
            bias_sb = persist.tile([P, C // P], F32)
            wq_sb = persist.tile([P, NCC, HD], MM_DT, tag="wq")
            wk_sb = persist.tile([P, NCC, HD], MM_DT, tag="wk")
            wv_sb = persist.tile([P, NCC, HD], MM_DT, tag="wv")
            wpT_sb = persist.tile([P, HD // P, C], MM_DT, tag="wpT")
            masks_sb = persist.tile([P, 4, 512], MM_DT, tag="masks")
            comboT = persist.tile([P, HD // P, T], MM_DT, tag="comboT")

            nc.sync.dma_start(bias_sb[:], bias[:].rearrange("(db p) -> p db", p=P))
            # ones column for the softmax-denominator row of the PV matmul
            nc.sync.dma_start(vaug[:, :, :, HS : HS + 1], onescol[:, :, :, None])
            nc.sync.dma_start(wq_sb[:], wq[:].rearrange("(co p) n -> p co n", p=P))
            nc.sync.dma_start(wk_sb[:], wk[:].rearrange("(co p) n -> p co n", p=P))
            nc.sync.dma_start(wv_sb[:], wv[:].rearrange("(co p) n -> p co n", p=P))
            nc.sync.dma_start(masks_sb[:], masks[:].rearrange("i k q -> k i q"))
            nc.sync.dma_start(wpT_sb[:], wpT[:].rearrange("(co p) n -> p co n", p=P))

            # ---- Phase 1: QKV projections ----
            for tb in range(NT):
                tsl = slice(tb * 512, (tb + 1) * 512)
                xt = xtp.tile([P, NCC, 512], MM_DT, tag="xt")
                nc.sync.dma_start(
                    xt[:], xT[:].rearrange("(co p) t -> p co t", p=P)[:, :, tsl]
                )
                for hb in range(HL // 2):
                    hsl = slice(hb * P, (hb + 1) * P)
                    for w_sb, dst in ((wq_sb, qt), (wk_sb, kt)):
                        pqk = ps_gen.tile([P, 512], F32, tag="pqk")
                        for co in range(NCC):
                            nc.tensor.matmul(
                                pqk[:],
                                w_sb[:, co, hsl],
                                xt[:, co, :],
                                start=(co == 0),
                                stop=(co == NCC - 1),
                            )
                        nc.vector.tensor_copy(out=dst[:, hb, tsl], in_=pqk[:])
                for ts2 in range(4):
                    pv = ps_gen.tile([P, 512], F32, tag="pqk")
                    for co in range(NCC):
                        nc.tensor.matmul(
                            pv[:],
                            xt[:, co, ts2 * P : (ts2 + 1) * P],
                            wv_sb[:, co, :],
                            start=(co == 0),
                            stop=(co == NCC - 1),
                        )
                    kc = tb * 4 + ts2
                    nc.vector.tensor_copy(
                        out=vaug[:, kc, :, 0:HS],
                        in_=pv[:].rearrange("p (h d) -> p h d", h=HL),
                    )

            # ---- Phase 2: causal attention ----
            # two persistent 2-bank score psums, ping-ponged so the PE can
            # compute the next group's scores while ACT exps the previous;
            # diagonal-suffix matmuls intentionally leave stale (bounded)
            # data in masked columns — same-tensor reuse keeps that visible
            # to the dep tracker
            pss2 = [
                ps_s.tile([P, 2, 512], F32, tag=f"pss{j}", name=f"pss{j}")
                for j in range(2)
            ]
            for h in range(HL):
                r0 = 64 * (h % 2)
                hp = h // 2
                for qb in range(NT):
                    q0 = qb * 512
                    po = ps_o.tile([P, 512], F32, tag="po")
                    last_kc = qb * 4 + 3
                    for g in range(2 * (qb + 1)):
                        pss = pss2[g % 2]
                        for i in range(2):
                            kc = 2 * g + i
                            mi = kc - 4 * qb
                            # diagonal tiles: columns q < kc*128-q0 are fully
                            # masked; skip them (psum there holds bounded
                            # stale scores — exp'd then zeroed by the mask).
                            # Only when qb>0 so the first-ever write of each
                            # bank is full-width.
                            c0 = kc * P - q0 if (mi >= 0 and qb > 0) else 0
                            nc.tensor.matmul(
                                pss[:, i, c0:512],
                                kt[r0 : r0 + 64, hp, kc * P : (kc + 1) * P],
                                qt[r0 : r0 + 64, hp, q0 + c0 : q0 + 512],
                                start=True,
                                stop=True,
                                tile_position=(r0, 0),
                            )
                        pt = ptp.tile([P, 2, 512], MM_DT, tag="pt")
                        nc.scalar.activation(pt[:], pss[:], Exp, scale=EXP_SCALE)
                        for i in range(2):
                            kc = 2 * g + i
                            mi = kc - 4 * qb
                            if mi >= 0:
                                nc.vector.tensor_mul(
                                    out=pt[:, i, :],
                                    in0=pt[:, i, :],
                                    in1=masks_sb[:, mi, :],
                                )
                        for i in range(2):
                            kc = 2 * g + i
                            mi = kc - 4 * qb
                            # diagonal tiles: PT columns q < kc*128-q0 are
                            # zero (masked) — skip accumulating them
                            c0 = max(0, kc * P - q0) if mi >= 0 else 0
                            nc.tensor.matmul(
                                po[0 : HS + 1, c0:512],
                                vaug[:, kc, h, :],
                                pt[:, i, c0:512],
                                start=(kc == 0),
                                stop=(kc == last_kc),
                            )
                    # normalize rows by the denominator row (65th) of po;
                    # custom-DVE reciprocal requires partition-0 input, so
                    # stage the row via an ACT copy first
                    den = misc.tile([1, 512], F32, tag="den")
                    nc.scalar.copy(out=den[:], in_=po[HS : HS + 1, :])
                    rc = misc.tile([1, 512], F32, tag="rc")
                    nc.vector.reciprocal_approx_fast(rc[:], den[:])
                    rb = misc.tile([HS, 512], F32, tag="rb")
                    nc.gpsimd.partition_broadcast(rb[:], rc[:])
                    nc.vector.tensor_mul(
                        out=comboT[(h % 2) * 64 : (h % 2) * 64 + 64, h // 2, q0 : q0 + 512],
                        in0=po[0:HS, :],
                        in1=rb[:],
                    )

            # ---- Phase 3: partial output projection: out^T [d 128, t 512] ----
            for tb in range(NT):
                tsl = slice(tb * 512, (tb + 1) * 512)
                for db in range(C // P):
                    pp = ps_gen.tile([P, 512], F32, tag="pqk")
                    for co in range(HD // P):
                        nc.tensor.matmul(
                            pp[:],
                            wpT_sb[:, co, db * P : (db + 1) * P],
                            comboT[:, co, tsl],
                            start=(co == 0),
                            stop=(co == HD // P - 1),
                        )
                    ot = outp.tile([P, 512], F32, tag="ot")
                    nc.vector.tensor_scalar_add(ot[:], pp[:], bias_sb[:, db : db + 1])
                    nc.sync.dma_start(outT[db * P : (db + 1) * P, tsl], ot[:])

    nc.finalize()
    return nc


def _causal_masks():
    # mask_i[kl, ql] = 1.0 iff (128*i + kl) <= ql, for the 4 diagonal key
    # chunks of a 512-wide query block (applied multiplicatively post-exp)
    kl = np.arange(P)[None, :, None]
    ql = np.arange(512)[None, None, :]
    i = np.arange(4)[:, None, None]
    return ((P * i + kl) <= ql).astype(np.float32)


def _in_maps(x, Wq, Wk, Wv, Wproj, bproj):
    masks = _causal_masks()
    zeros_bias = np.zeros_like(bproj)
    onescol = np.ones((P, NKC, HL), dtype=MM_NP)
    maps = []
    for core in range(8):
        b, hg = core // 2, core % 2
        hs = slice(hg * HL, (hg + 1) * HL)
        maps.append(
            {
                "xT": np.ascontiguousarray(x[b].T).astype(MM_NP),
                "wq": np.ascontiguousarray(
                    Wq[hs].transpose(1, 0, 2).reshape(C, HD).astype(MM_NP)
                ),
                "wk": np.ascontiguousarray(
                    Wk[hs].transpose(1, 0, 2).reshape(C, HD).astype(MM_NP)
                ),
                "wv": np.ascontiguousarray(
                    Wv[hs].transpose(1, 0, 2).reshape(C, HD).astype(MM_NP)
                ),
                "wpT": np.ascontiguousarray(Wproj[:, hg * HD : (hg + 1) * HD].T).astype(MM_NP),
                "bias": np.ascontiguousarray(bproj if hg == 0 else zeros_bias),
                "masks": masks.astype(MM_NP),
                "onescol": onescol,
            }
        )
    return maps


def get_program():
    global _PROGRAM
    if _PROGRAM is None:
        _PROGRAM = _build_program()
    return _PROGRAM


def kernel(x, Wq, Wk, Wv, Wproj, bproj, _run_kwargs=None):
    x = np.asarray(x, dtype=np.float32)
    Wq = np.asarray(Wq, dtype=np.float32)
    Wk = np.asarray(Wk, dtype=np.float32)
    Wv = np.asarray(Wv, dtype=np.float32)
    Wproj = np.asarray(Wproj, dtype=np.float32)
    bproj = np.asarray(bproj, dtype=np.float32)

    nc = get_program()
    res = run_bass_kernel_spmd(
        nc,
        _in_maps(x, Wq, Wk, Wv, Wproj, bproj),
        core_ids=list(range(8)),
        **(_run_kwargs or {}),
    )
    out = np.empty((B, T, C), dtype=np.float32)
    for b in range(B):
        out[b] = (res.results[2 * b]["outT"] + res.results[2 * b + 1]["outT"]).T
    if _run_kwargs:
        kernel.last_results = res
    return out


# revision 27
# speedup vs baseline: 1.3637x; 1.0038x over previous
"""Multi-head causal attention (B=4,T=2048,C=1024,H=16,HS=64) on 8 TRN2 cores.

Sharding: core c -> batch b=c//2, head-group hg=c%2 (8 heads each).
Each core computes QKV projections for its heads, causal flash-attention,
and a partial output projection over its 512 combo channels, emitting
out^T partial [1024, 2048].  Host sums the two partials per batch (the
tensor-parallel all-reduce) and transposes.

Matmuls run in bf16 (fp32 PSUM accumulation); softmax skips
max-subtraction (scores ~ N(0,1), exp never overflows); the softmax
denominator comes free as a 65th row of the PV matmul via a ones-column
appended to V.  The kernel is a single software pipeline over the four
512-query blocks: QKV(tb) -> attention(qb=tb, all heads) -> proj(tb).
"""

import os
import sys

if "/opt/trn_rl_repo" not in sys.path:
    sys.path.insert(0, "/opt/trn_rl_repo")

import ml_dtypes
import numpy as np

import concourse.bass as bass
import concourse.mybir as mybir
import concourse.tile as tile
from concourse import bacc
from concourse.bass_utils import run_bass_kernel_spmd

P = 128
B, T, C, H = 4, 2048, 1024, 16
HS = C // H              # 64
HL = H // 2              # 8 local heads per core
HD = HL * HS             # 512 local combo channels
NT = T // 512            # 4 query blocks of 512
NCC = C // P             # 8 contraction chunks over C
NKC = T // P             # 16 key chunks of 128
F32 = mybir.dt.float32
F32R = mybir.dt.float32r
BF16 = mybir.dt.bfloat16
EXP_SCALE = float(HS) ** -0.5  # 1/8, folded into the exp activation

# matmul input dtype: bf16 (full PE rate @2.4GHz) or f32r (~1.2GHz, 10x tighter)
MM_DT = {"bf16": BF16, "f32r": F32R}[os.environ.get("MM_DT", "bf16")]
MM_NP = {BF16: ml_dtypes.bfloat16, F32R: np.float32}[MM_DT]

_PROGRAM = None


def _build_program():
    nc = bacc.Bacc("TRN2", target_bir_lowering=False, debug=False, num_devices=8)

    xT = nc.dram_tensor("xT", [C, T], MM_DT, kind="ExternalInput")
    wq = nc.dram_tensor("wq", [C, HD], MM_DT, kind="ExternalInput")
    wk = nc.dram_tensor("wk", [C, HD], MM_DT, kind="ExternalInput")
    wv = nc.dram_tensor("wv", [C, HD], MM_DT, kind="ExternalInput")
    wpT = nc.dram_tensor("wpT", [HD, C], MM_DT, kind="ExternalInput")
    bias = nc.dram_tensor("bias", [C], F32, kind="ExternalInput")
    masks = nc.dram_tensor("masks", [4, P, 512], MM_DT, kind="ExternalInput")
    onescol = nc.dram_tensor("onescol", [P, NKC, HL], MM_DT, kind="ExternalInput")
    outT = nc.dram_tensor("outT", [C, T], F32, kind="ExternalOutput")

    Exp = mybir.ActivationFunctionType.Exp

    with tile.TileContext(nc) as tc:
        with (
            tc.tile_pool(name="persist", bufs=1) as persist,
            tc.tile_pool(name="xtp", bufs=2) as xtp,
                        tc.tile_pool(name="ptp", bufs=2) as ptp,
            tc.tile_pool(name="misc", bufs=2) as misc,
            tc.tile_pool(name="outp", bufs=3) as outp,
            tc.tile_pool(name="ps_gen", bufs=2, space="PSUM") as ps_gen,
            tc.tile_pool(name="ps_s", bufs=1, space="PSUM") as ps_s,
            tc.tile_pool(name="ps_o", bufs=2, space="PSUM") as ps_o,
        ):
            # Q^T / K^T with head pairs stacked on partitions: chunk j holds
            # head 2j in rows 0-63 and head 2j+1 in rows 64-127
            qt = persist.tile([P, HL // 2, T], MM_DT)
            kt = persist.tile([P, HL // 2, T], MM_DT)
            vaug = persist.tile([P, NKC, HL, HS + 1], MM_DT)
            bias_sb = persist.tile([P, C // P], F32)
            wq_sb = persist.tile([P, NCC, HD], MM_DT, tag="wq")
            wk_sb = persist.tile([P, NCC, HD], MM_DT, tag="wk")
            wv_sb = persist.tile([P, NCC, HD], MM_DT, tag="wv")
            wpT_sb = persist.tile([P, HD // P, C], MM_DT, tag="wpT")
            masks_sb = persist.tile([P, 4, 512], MM_DT, tag="masks")
            comboT = persist.tile([P, HD // P, T], MM_DT, tag="comboT")

            nc.sync.dma_start(bias_sb[:], bias[:].rearrange("(db p) -> p db", p=P))
            # ones column for the softmax-denominator row of the PV matmul
            nc.sync.dma_start(vaug[:, :, :, HS : HS + 1], onescol[:, :, :, None])
            for _h in range(2):
                nc.sync.dma_start(wq_sb[:, 4 * _h : 4 * _h + 4, :], wq[:].rearrange("(co p) n -> p co n", p=P)[:, 4 * _h : 4 * _h + 4, :])
            for _h in range(2):
                nc.sync.dma_start(wk_sb[:, 4 * _h : 4 * _h + 4, :], wk[:].rearrange("(co p) n -> p co n", p=P)[:, 4 * _h : 4 * _h + 4, :])
            for _h in range(2):
                nc.sync.dma_start(wv_sb[:, 4 * _h : 4 * _h + 4, :], wv[:].rearrange("(co p) n -> p co n", p=P)[:, 4 * _h : 4 * _h + 4, :])
            nc.sync.dma_start(masks_sb[:], masks[:].rearrange("i k q -> k i q"))
            nc.sync.dma_start(wpT_sb[:], wpT[:].rearrange("(co p) n -> p co n", p=P))

            # ---- Phase 1: QKV projections ----
            for tb in range(NT):
                tsl = slice(tb * 512, (tb + 1) * 512)
                xt = xtp.tile([P, NCC, 512], MM_DT, tag="xt")
                nc.sync.dma_start(
                    xt[:], xT[:].rearrange("(co p) t -> p co t", p=P)[:, :, tsl]
                )
                for hb in range(HL // 2):
                    hsl = slice(hb * P, (hb + 1) * P)
                    for w_sb, dst in ((wq_sb, qt), (wk_sb, kt)):
                        pqk = ps_gen.tile([P, 512], F32, tag="pqk")
                        for co in range(NCC):
                            nc.tensor.matmul(
                                pqk[:],
                                w_sb[:, co, hsl],
                                xt[:, co, :],
                                start=(co == 0),
                                stop=(co == NCC - 1),
                            )
                        nc.vector.tensor_copy(out=dst[:, hb, tsl], in_=pqk[:])
                for ts2 in range(4):
                    pv = ps_gen.tile([P, 512], F32, tag="pqk")
                    for co in range(NCC):
                        nc.tensor.matmul(
                            pv[:],
                            xt[:, co, ts2 * P : (ts2 + 1) * P],
                            wv_sb[:, co, :],
                            start=(co == 0),
                            stop=(co == NCC - 1),
                        )
                    kc = tb * 4 + ts2
                    nc.vector.tensor_copy(
                        out=vaug[:, kc, :, 0:HS],
                        in_=pv[:].rearrange("p (h d) -> p h d", h=HL),
                    )

            # ---- Phase 2: causal attention ----
            # two persistent 2-bank score psums, ping-ponged so the PE can
            # compute the next group's scores while ACT exps the previous;
            # diagonal-suffix matmuls intentionally leave stale (bounded)
            # data in masked columns — same-tensor reuse keeps that visible
            # to the dep tracker
            pss2 = [
                ps_s.tile([P, 2, 512], F32, tag=f"pss{j}", name=f"pss{j}")
                for j in range(2)
            ]
            for qb in range(NT):
                q0 = qb * 512
                for h in range(HL):
                    r0 = 64 * (h % 2)
                    hp = h // 2
                    po = ps_o.tile([P, 512], F32, tag="po")
                    last_kc = qb * 4 + 3
                    for g in range(2 * (qb + 1)):
                        pss = pss2[g % 2]
                        for i in range(2):
                            kc = 2 * g + i
                            mi = kc - 4 * qb
                            # diagonal tiles: columns q < kc*128-q0 are fully
                            # masked; skip them (psum there holds bounded
                            # stale scores — exp'd then zeroed by the mask).
                            # Only when qb>0 so the first-ever write of each
                            # bank is full-width.
                            c0 = kc * P - q0 if (mi >= 0 and qb > 0) else 0
                            nc.tensor.matmul(
                                pss[:, i, c0:512],
                                kt[r0 : r0 + 64, hp, kc * P : (kc + 1) * P],
                                qt[r0 : r0 + 64, hp, q0 + c0 : q0 + 512],
                                start=True,
                                stop=True,
                                tile_position=(r0, 0),
                            )
                        pt = ptp.tile([P, 2, 512], MM_DT, tag="pt")
                        nc.scalar.activation(pt[:], pss[:], Exp, scale=EXP_SCALE)
                        for i in range(2):
                            kc = 2 * g + i
                            mi = kc - 4 * qb
                            if mi >= 0:
                                nc.vector.tensor_mul(
                                    out=pt[:, i, :],
                                    in0=pt[:, i, :],
                                    in1=masks_sb[:, mi, :],
                                )
                        for i in range(2):
                            kc = 2 * g + i
                            mi = kc - 4 * qb
                            # diagonal tiles: PT columns q < kc*128-q0 are
                            # zero (masked) — skip accumulating them
                            c0 = max(0, kc * P - q0) if mi >= 0 else 0
                            nc.tensor.matmul(
                                po[0 : HS + 1, c0:512],
                                vaug[:, kc, h, :],
                                pt[:, i, c0:512],
                                start=(kc == 0),
                                stop=(kc == last_kc),
                            )
                    # normalize rows by the denominator row (65th) of po;
                    # custom-DVE reciprocal requires partition-0 input, so
                    # stage the row via an ACT copy first
                    den = misc.tile([1, 512], F32, tag="den")
                    nc.scalar.copy(out=den[:], in_=po[HS : HS + 1, :])
                    rc = misc.tile([1, 512], F32, tag="rc")
                    nc.vector.reciprocal_approx_fast(rc[:], den[:])
                    rb = misc.tile([HS, 512], F32, tag="rb")
                    nc.gpsimd.partition_broadcast(rb[:], rc[:])
                    nc.vector.tensor_mul(
                        out=comboT[(h % 2) * 64 : (h % 2) * 64 + 64, h // 2, q0 : q0 + 512],
                        in0=po[0:HS, :],
                        in1=rb[:],
                    )
                # ---- partial output projection for this q-block ----
                for db in range(C // P):
                    pp = ps_gen.tile([P, 512], F32, tag="pqk")
                    for co in range(HD // P):
                        nc.tensor.matmul(
                            pp[:],
                            wpT_sb[:, co, db * P : (db + 1) * P],
                            comboT[:, co, q0 : q0 + 512],
                            start=(co == 0),
                            stop=(co == HD // P - 1),
                        )
                    ot = outp.tile([P, 512], F32, tag="ot")
                    nc.vector.tensor_scalar_add(ot[:], pp[:], bias_sb[:, db : db + 1])
                    nc.sync.dma_start(outT[db * P : (db + 1) * P, q0 : q0 + 512], ot[:])


    nc.finalize()
    return nc


def _causal_masks():
    # mask_i[kl, ql] = 1.0 iff (128*i + kl) <= ql, for the 4 diagonal key
    # chunks of a 512-wide query block (applied multiplicatively post-exp)
    kl = np.arange(P)[None, :, None]
    ql = np.arange(512)[None, None, :]
    i = np.arange(4)[:, None, None]
    return ((P * i + kl) <= ql).astype(np.float32)


def _in_maps(x, Wq, Wk, Wv, Wproj, bproj):
    masks = _causal_masks()
    zeros_bias = np.zeros_like(bproj)
    onescol = np.ones((P, NKC, HL), dtype=MM_NP)
    maps = []
    for core in range(8):
        b, hg = core // 2, core % 2
        hs = slice(hg * HL, (hg + 1) * HL)
        maps.append(
            {
                "xT": np.ascontiguousarray(x[b].T).astype(MM_NP),
                "wq": np.ascontiguousarray(
                    Wq[hs].transpose(1, 0, 2).reshape(C, HD).astype(MM_NP)
                ),
                "wk": np.ascontiguousarray(
                    Wk[hs].transpose(1, 0, 2).reshape(C, HD).astype(MM_NP)
                ),
                "wv": np.ascontiguousarray(
                    Wv[hs].transpose(1, 0, 2).reshape(C, HD).astype(MM_NP)
                ),
                "wpT": np.ascontiguousarray(Wproj[:, hg * HD : (hg + 1) * HD].T).astype(MM_NP),
                "bias": np.ascontiguousarray(bproj if hg == 0 else zeros_bias),
                "masks": masks.astype(MM_NP),
                "onescol": onescol,
            }
        )
    return maps


def get_program():
    global _PROGRAM
    if _PROGRAM is None:
        _PROGRAM = _build_program()
    return _PROGRAM


def kernel(x, Wq, Wk, Wv, Wproj, bproj, _run_kwargs=None):
    x = np.asarray(x, dtype=np.float32)
    Wq = np.asarray(Wq, dtype=np.float32)
    Wk = np.asarray(Wk, dtype=np.float32)
    Wv = np.asarray(Wv, dtype=np.float32)
    Wproj = np.asarray(Wproj, dtype=np.float32)
    bproj = np.asarray(bproj, dtype=np.float32)

    nc = get_program()
    res = run_bass_kernel_spmd(
        nc,
        _in_maps(x, Wq, Wk, Wv, Wproj, bproj),
        core_ids=list(range(8)),
        **(_run_kwargs or {}),
    )
    out = np.empty((B, T, C), dtype=np.float32)
    for b in range(B):
        out[b] = (res.results[2 * b]["outT"] + res.results[2 * b + 1]["outT"]).T
    if _run_kwargs:
        kernel.last_results = res
    return out


# revision 29
# speedup vs baseline: 1.4309x; 1.0493x over previous
"""Multi-head causal attention (B=4,T=2048,C=1024,H=16,HS=64) on 8 TRN2 cores.

Sharding: core c -> batch b=c//2, head-group hg=c%2 (8 heads each).
Each core computes QKV projections for its heads, causal flash-attention,
and a partial output projection over its 512 combo channels, emitting
out^T partial [1024, 2048].  Host sums the two partials per batch (the
tensor-parallel all-reduce) and transposes.

Matmuls run in bf16 (fp32 PSUM accumulation); softmax skips
max-subtraction (scores ~ N(0,1), exp never overflows); the softmax
denominator comes free as a 65th row of the PV matmul via a ones-column
appended to V.  The kernel is a single software pipeline over the four
512-query blocks: QKV(tb) -> attention(qb=tb, all heads) -> proj(tb).
"""

import os
import sys

if "/opt/trn_rl_repo" not in sys.path:
    sys.path.insert(0, "/opt/trn_rl_repo")

import ml_dtypes
import numpy as np

import concourse.bass as bass
import concourse.mybir as mybir
import concourse.tile as tile
from concourse import bacc
from concourse.bass_utils import run_bass_kernel_spmd

P = 128
B, T, C, H = 4, 2048, 1024, 16
HS = C // H              # 64
HL = H // 2              # 8 local heads per core
HD = HL * HS             # 512 local combo channels
NT = T // 512            # 4 query blocks of 512
NCC = C // P             # 8 contraction chunks over C
NKC = T // P             # 16 key chunks of 128
F32 = mybir.dt.float32
F32R = mybir.dt.float32r
BF16 = mybir.dt.bfloat16
EXP_SCALE = float(HS) ** -0.5  # 1/8, folded into the exp activation

# matmul input dtype: bf16 (full PE rate @2.4GHz) or f32r (~1.2GHz, 10x tighter)
MM_DT = {"bf16": BF16, "f32r": F32R}[os.environ.get("MM_DT", "bf16")]
MM_NP = {BF16: ml_dtypes.bfloat16, F32R: np.float32}[MM_DT]

_PROGRAM = None


def _build_program():
    nc = bacc.Bacc("TRN2", target_bir_lowering=False, debug=False, num_devices=8)

    xT = nc.dram_tensor("xT", [C, T], MM_DT, kind="ExternalInput")
    wq = nc.dram_tensor("wq", [C, HD], MM_DT, kind="ExternalInput")
    wk = nc.dram_tensor("wk", [C, HD], MM_DT, kind="ExternalInput")
    wv = nc.dram_tensor("wv", [C, HD], MM_DT, kind="ExternalInput")
    wpT = nc.dram_tensor("wpT", [HD, C], MM_DT, kind="ExternalInput")
    bias = nc.dram_tensor("bias", [C], F32, kind="ExternalInput")
    masks = nc.dram_tensor("masks", [4, P, 512], MM_DT, kind="ExternalInput")
    outT = nc.dram_tensor("outT", [C, T], F32, kind="ExternalOutput")

    Exp = mybir.ActivationFunctionType.Exp

    with tile.TileContext(nc) as tc:
        with (
            tc.tile_pool(name="persist", bufs=1) as persist,
            tc.tile_pool(name="xtp", bufs=2) as xtp,
                        tc.tile_pool(name="ptp", bufs=2) as ptp,
            tc.tile_pool(name="misc", bufs=2) as misc,
            tc.tile_pool(name="outp", bufs=3) as outp,
            tc.tile_pool(name="ps_gen", bufs=2, space="PSUM") as ps_gen,
            tc.tile_pool(name="ps_s", bufs=1, space="PSUM") as ps_s,
            tc.tile_pool(name="ps_o", bufs=2, space="PSUM") as ps_o,
        ):
            # Q^T / K^T with head pairs stacked on partitions: chunk j holds
            # head 2j in rows 0-63 and head 2j+1 in rows 64-127
            qt = persist.tile([P, HL // 2, T], MM_DT)
            kt = persist.tile([P, HL // 2, T], MM_DT)
            vaug = persist.tile([P, NKC, HL, HS + 1], MM_DT)
            bias_sb = persist.tile([P, C // P], F32)
            wq_sb = persist.tile([P, NCC, HD], MM_DT, tag="wq")
            wk_sb = persist.tile([P, NCC, HD], MM_DT, tag="wk")
            wv_sb = persist.tile([P, NCC, HD], MM_DT, tag="wv")
            wpT_sb = persist.tile([P, HD // P, C], MM_DT, tag="wpT")
            masks_sb = persist.tile([P, 4, 512], MM_DT, tag="masks")
            comboT = persist.tile([P, HD // P, T], MM_DT, tag="comboT")

            # ones column for the softmax-denominator row of the PV matmul
            nc.vector.memset(vaug[:, :, :, HS : HS + 1], 1.0)
            # setup loads spread across engine DMA queues so the first
            # matmuls (needing wq/wk chunk 0 + the first x^T block) can
            # start as early as possible
            for _h in range(2):
                nc.scalar.dma_start(wq_sb[:, 4 * _h : 4 * _h + 4, :], wq[:].rearrange("(co p) n -> p co n", p=P)[:, 4 * _h : 4 * _h + 4, :])
            for _h in range(2):
                nc.gpsimd.dma_start(wk_sb[:, 4 * _h : 4 * _h + 4, :], wk[:].rearrange("(co p) n -> p co n", p=P)[:, 4 * _h : 4 * _h + 4, :])
            for _h in range(2):
                nc.scalar.dma_start(wv_sb[:, 4 * _h : 4 * _h + 4, :], wv[:].rearrange("(co p) n -> p co n", p=P)[:, 4 * _h : 4 * _h + 4, :])
            nc.gpsimd.dma_start(masks_sb[:], masks[:].rearrange("i k q -> k i q"))
            nc.gpsimd.dma_start(wpT_sb[:], wpT[:].rearrange("(co p) n -> p co n", p=P))
            nc.gpsimd.dma_start(bias_sb[:], bias[:].rearrange("(db p) -> p db", p=P))

            # ---- Phase 1: QKV projections ----
            for tb in range(NT):
                tsl = slice(tb * 512, (tb + 1) * 512)
                xt = xtp.tile([P, NCC, 512], MM_DT, tag="xt")
                nc.sync.dma_start(
                    xt[:], xT[:].rearrange("(co p) t -> p co t", p=P)[:, :, tsl]
                )
                for hb in range(HL // 2):
                    hsl = slice(hb * P, (hb + 1) * P)
                    for w_sb, dst in ((wq_sb, qt), (wk_sb, kt)):
                        pqk = ps_gen.tile([P, 512], F32, tag="pqk")
                        for co in range(NCC):
                            nc.tensor.matmul(
                                pqk[:],
                                w_sb[:, co, hsl],
                                xt[:, co, :],
                                start=(co == 0),
                                stop=(co == NCC - 1),
                            )
                        nc.vector.tensor_copy(out=dst[:, hb, tsl], in_=pqk[:])
                for ts2 in range(4):
                    pv = ps_gen.tile([P, 512], F32, tag="pqk")
                    for co in range(NCC):
                        nc.tensor.matmul(
                            pv[:],
                            xt[:, co, ts2 * P : (ts2 + 1) * P],
                            wv_sb[:, co, :],
                            start=(co == 0),
                            stop=(co == NCC - 1),
                        )
                    kc = tb * 4 + ts2
                    nc.vector.tensor_copy(
                        out=vaug[:, kc, :, 0:HS],
                        in_=pv[:].rearrange("p (h d) -> p h d", h=HL),
                    )

            # ---- Phase 2: causal attention ----
            # two persistent 2-bank score psums, ping-ponged so the PE can
            # compute the next group's scores while ACT exps the previous;
            # diagonal-suffix matmuls intentionally leave stale (bounded)
            # data in masked columns — same-tensor reuse keeps that visible
            # to the dep tracker
            pss2 = [
                ps_s.tile([P, 2, 512], F32, tag=f"pss{j}", name=f"pss{j}")
                for j in range(2)
            ]
            for qb in range(NT):
                q0 = qb * 512
                for h in range(HL):
                    r0 = 64 * (h % 2)
                    hp = h // 2
                    po = ps_o.tile([P, 512], F32, tag="po")
                    last_kc = qb * 4 + 3
                    for g in range(2 * (qb + 1)):
                        pss = pss2[g % 2]
                        for i in range(2):
                            kc = 2 * g + i
                            mi = kc - 4 * qb
                            # diagonal tiles: columns q < kc*128-q0 are fully
                            # masked; skip them (psum there holds bounded
                            # stale scores — exp'd then zeroed by the mask).
                            # Only when qb>0 so the first-ever write of each
                            # bank is full-width.
                            c0 = kc * P - q0 if (mi >= 0 and qb > 0) else 0
                            nc.tensor.matmul(
                                pss[:, i, c0:512],
                                kt[r0 : r0 + 64, hp, kc * P : (kc + 1) * P],
                                qt[r0 : r0 + 64, hp, q0 + c0 : q0 + 512],
                                start=True,
                                stop=True,
                                tile_position=(r0, 0),
                            )
                        pt = ptp.tile([P, 2, 512], MM_DT, tag="pt")
                        nc.scalar.activation(pt[:], pss[:], Exp, scale=EXP_SCALE)
                        for i in range(2):
                            kc = 2 * g + i
                            mi = kc - 4 * qb
                            if mi >= 0:
                                nc.vector.tensor_mul(
                                    out=pt[:, i, :],
                                    in0=pt[:, i, :],
                                    in1=masks_sb[:, mi, :],
                                )
                        for i in range(2):
                            kc = 2 * g + i
                            mi = kc - 4 * qb
                            # diagonal tiles: PT columns q < kc*128-q0 are
                            # zero (masked) — skip accumulating them
                            c0 = max(0, kc * P - q0) if mi >= 0 else 0
                            nc.tensor.matmul(
                                po[0 : HS + 1, c0:512],
                                vaug[:, kc, h, :],
                                pt[:, i, c0:512],
                                start=(kc == 0),
                                stop=(kc == last_kc),
                            )
                    # normalize rows by the denominator row (65th) of po;
                    # custom-DVE reciprocal requires partition-0 input, so
                    # stage the row via an ACT copy first
                    den = misc.tile([1, 512], F32, tag="den")
                    nc.scalar.copy(out=den[:], in_=po[HS : HS + 1, :])
                    rc = misc.tile([1, 512], F32, tag="rc")
                    nc.vector.reciprocal_approx_fast(rc[:], den[:])
                    rb = misc.tile([HS, 512], F32, tag="rb")
                    nc.gpsimd.partition_broadcast(rb[:], rc[:])
                    nc.vector.tensor_mul(
                        out=comboT[(h % 2) * 64 : (h % 2) * 64 + 64, h // 2, q0 : q0 + 512],
                        in0=po[0:HS, :],
                        in1=rb[:],
                    )
                # ---- partial output projection for this q-block ----
                for db in range(C // P):
                    pp = ps_gen.tile([P, 512], F32, tag="pqk")
                    for co in range(HD // P):
                        nc.tensor.matmul(
                            pp[:],
                            wpT_sb[:, co, db * P : (db + 1) * P],
                            comboT[:, co, q0 : q0 + 512],
                            start=(co == 0),
                            stop=(co == HD // P - 1),
                        )
                    ot = outp.tile([P, 512], F32, tag="ot")
                    nc.vector.tensor_scalar_add(ot[:], pp[:], bias_sb[:, db : db + 1])
                    nc.sync.dma_start(outT[db * P : (db + 1) * P, q0 : q0 + 512], ot[:])


    nc.finalize()
    return nc


def _causal_masks():
    # mask_i[kl, ql] = 1.0 iff (128*i + kl) <= ql, for the 4 diagonal key
    # chunks of a 512-wide query block (applied multiplicatively post-exp)
    kl = np.arange(P)[None, :, None]
    ql = np.arange(512)[None, None, :]
    i = np.arange(4)[:, None, None]
    return ((P * i + kl) <= ql).astype(np.float32)


def _in_maps(x, Wq, Wk, Wv, Wproj, bproj):
    masks = _causal_masks()
    zeros_bias = np.zeros_like(bproj)
    maps = []
    for core in range(8):
        b, hg = core // 2, core % 2
        hs = slice(hg * HL, (hg + 1) * HL)
        maps.append(
            {
                "xT": np.ascontiguousarray(x[b].T).astype(MM_NP),
                "wq": np.ascontiguousarray(
                    Wq[hs].transpose(1, 0, 2).reshape(C, HD).astype(MM_NP)
                ),
                "wk": np.ascontiguousarray(
                    Wk[hs].transpose(1, 0, 2).reshape(C, HD).astype(MM_NP)
                ),
                "wv": np.ascontiguousarray(
                    Wv[hs].transpose(1, 0, 2).reshape(C, HD).astype(MM_NP)
                ),
                "wpT": np.ascontiguousarray(Wproj[:, hg * HD : (hg + 1) * HD].T).astype(MM_NP),
                "bias": np.ascontiguousarray(bproj if hg == 0 else zeros_bias),
                "masks": masks.astype(MM_NP),
            }
        )
    return maps


def get_program():
    global _PROGRAM
    if _PROGRAM is None:
        _PROGRAM = _build_program()
    return _PROGRAM


def kernel(x, Wq, Wk, Wv, Wproj, bproj, _run_kwargs=None):
    x = np.asarray(x, dtype=np.float32)
    Wq = np.asarray(Wq, dtype=np.float32)
    Wk = np.asarray(Wk, dtype=np.float32)
    Wv = np.asarray(Wv, dtype=np.float32)
    Wproj = np.asarray(Wproj, dtype=np.float32)
    bproj = np.asarray(bproj, dtype=np.float32)

    nc = get_program()
    res = run_bass_kernel_spmd(
        nc,
        _in_maps(x, Wq, Wk, Wv, Wproj, bproj),
        core_ids=list(range(8)),
        **(_run_kwargs or {}),
    )
    out = np.empty((B, T, C), dtype=np.float32)
    for b in range(B):
        out[b] = (res.results[2 * b]["outT"] + res.results[2 * b + 1]["outT"]).T
    if _run_kwargs:
        kernel.last_results = res
    return out
